# revision 1
# baseline (speedup 1.0000x reference)
"""Trainium2 Bass kernel for nn_MDCN (mixture-density head forward pass).

Reference computation (B=2048, F=1024, M=128):
    rho = tanh(feature @ h2rho_w.T + h2rho_b);  rho[:, 0] = 0.95
    pi  = softmax(feature @ h2pi_w.T + h2pi_b)
    var0 = exp(feature @ h2var_w.T + h2var_b)
    var = (1 - exp(rho)) * var0 + 1e-4
    W_ = r*muW + s*(r*(zstd/wstd)*(W-muW) + Z*s),  s = sqrt(1-r^2)
    mu = einsum('bmf,bf->bm', W_, feature)

Key algebraic collapse: with a = (zstd/wstd)*(W-muW),
    mu[b,m] = r*d1[b] + r*s*d2[b] + s^2*d3[b]
  where d1 = feature@muW, d2 = feature@a, d3 = feature@Z.
So the [B,M,F] einsum becomes 3 extra columns of one fused matmul:
    logits[b, 0:387] = feature[b] @ [wrho.T | wpi.T | wvar.T | muW | a | Z]
Additionally s = sqrt(1-r^2) = sech(u) = (1+tanh(u)) * exp(-u), so the whole
epilogue needs only Tanh and Exp (one ACT table set), and the clamped first
mixture column is a compile-time constant.

Sharding: pure data-parallel over batch across 8 cores (256 rows/core),
weights replicated. No collectives needed (forward only).
"""

import os
from contextlib import ExitStack

import numpy as np

import concourse.bass as bass
import concourse.bacc as bacc
import concourse.mybir as mybir
import concourse.tile as tile
from concourse.bass_utils import run_bass_kernel_spmd

B, F, M = 2048, 1024, 128
NCORES = 8
BC = B // NCORES            # 256 batch rows per core
NT = BC // 128              # 2 partition tiles per core
KC = F // 128               # 8 contraction chunks
NW = 3 * M + 4              # 388 fused output columns (384 logits +
                            # 3 mu dot-products + 1 pad; fp32r matmul
                            # requires an even destination free-dim)
RHO_1 = np.float32(0.95)
TAU_INV = 1.0e-4
# s at the clamped column, computed exactly as the fp32 reference does:
# s0 = sqrt(1 - 0.95f * 0.95f)
S0 = float(np.sqrt(np.float32(1.0) - RHO_1 * RHO_1))

F32 = mybir.dt.float32
F32R = mybir.dt.float32r
F16 = mybir.dt.float16
AF = mybir.ActivationFunctionType
OP = mybir.AluOpType

# Matmul operand dtype. The per-core DMA path sustains only ~200 GB/s, so
# the kernel is input-bandwidth-bound and fp16 inputs halve its runtime.
# fp16 (11-bit mantissa) keeps the worst-case output error ~1e-3 of scale
# (vs 2.8e-4 for float32r, 4e-3 for bfloat16); accumulation is fp32 in PSUM.
# Set to F32R for a full-precision fallback (bit-compatible with fp32 but
# streams 1 row/cycle vs 4 for plain fp32).
MM_DT = F16
MM_NP = np.float16 if MM_DT == F16 else np.float32


def _emit_body(nc, tc, pools, fwc_dram, ft1_dram, blk_dram, out_dram):
    """Emit one full forward pass: DMA in -> fused matmul -> epilogue -> out."""
    consts, fwpool, psum, work = pools

    # Tiny bias block goes on the gpsimd (SWDGE) queue so it does not block
    # the head of the SP (HWDGE) queue that streams the big inputs.
    blk = consts.tile([1, 128 + NW], MM_DT, tag="bias_blk", name="bias_blk")
    nc.gpsimd.dma_start(blk[:], blk_dram)

    # Each dma_start carries ~0.5-2us of fixed cost on this part, so inputs
    # are consolidated into three transfers: two fused feature+weight halves
    # (PE starts on the first while the second streams) and tile-1 features.
    H = KC // 2
    fwcA = fwpool.tile([128, H, 128 + NW], MM_DT, tag="fwcA", name="fwcA")
    nc.sync.dma_start(fwcA[:], fwc_dram[0:H].rearrange("c p j -> p c j"))
    fwcB = fwpool.tile([128, H, 128 + NW], MM_DT, tag="fwcB", name="fwcB")
    nc.sync.dma_start(fwcB[:], fwc_dram[H:KC].rearrange("c p j -> p c j"))
    ft1 = fwpool.tile([128, KC, 128], MM_DT, tag="ft1", name="ft1")
    nc.sync.dma_start(ft1[:], ft1_dram)

    def fwc_c(c):
        return fwcA[:, c, :] if c < H else fwcB[:, c - H, :]

    # Fused matmul: psum[t][b, :] = bias + sum_c featT_c[:,b].T @ wcat_c
    pt = [psum.tile([128, NW], F32, tag=f"psum{t}", name=f"psum{t}")
          for t in range(NT)]
    for t in range(NT):
        nc.tensor.matmul(pt[t][:], blk[:, 0:128], blk[:, 128:128 + NW],
                         start=True, stop=False)
    for c in range(KC):
        nc.tensor.matmul(pt[0][:], fwc_c(c)[:, 0:128],
                         fwc_c(c)[:, 128:128 + NW],
                         start=False, stop=(c == KC - 1))
    for c in range(KC):
        nc.tensor.matmul(pt[1][:], ft1[:, c, :],
                         fwc_c(c)[:, 128:128 + NW],
                         start=False, stop=(c == KC - 1))

    # Epilogue per 128-row tile. Layout of psum P: [rho | pi | var | d1 d2 d3 0]
    for t in range(NT):
        P = pt[t][:]
        tg = f"t{t}"

        # psum layout (rho weights negated on host): [-u | pi | var | d 0].
        # r = tanh(u) = tanh(-1 * P[:,0:M]); one exp covers e^-u, e^pi, e^var.
        r = work.tile([128, M], F32, tag="r" + tg, name="r" + tg)
        nc.scalar.activation(r[:], P[:, 0:M], AF.Tanh, scale=-1.0)
        E = work.tile([128, 3 * M], F32, tag="E" + tg, name="E" + tg)
        nc.scalar.activation(E[:], P[:, 0:3 * M], AF.Exp)
        eneg, epi, var0 = E[:, 0:M], E[:, M:2 * M], E[:, 2 * M:3 * M]

        dsb = work.tile([128, 3], F32, tag="dsb" + tg, name="dsb" + tg)
        nc.vector.tensor_copy(dsb[:], P[:, 3 * M:3 * M + 3])

        # clamp first mixture BEFORE exp(rho) and the mu chain
        nc.vector.memset(r[:, 0:1], float(RHO_1))
        erho = work.tile([128, M], F32, tag="erho" + tg, name="erho" + tg)
        nc.scalar.activation(erho[:], r[:], AF.Exp)

        out_sb = work.tile([128, 3 * M], F32, tag="out" + tg, name="out" + tg)

        # s = (1 + r) * exp(-u) = sqrt(1 - r^2); fix clamped column
        s = work.tile([128, M], F32, tag="s" + tg, name="s" + tg)
        nc.vector.scalar_tensor_tensor(s[:], r[:], 1.0, eneg, OP.add, OP.mult)
        nc.vector.memset(s[:, 0:1], S0)

        # mu = r*(d1 + s*d2) + s^2*d3
        ss = work.tile([128, M], F32, tag="ss" + tg, name="ss" + tg)
        nc.vector.tensor_mul(ss[:], s[:], s[:])
        q = work.tile([128, M], F32, tag="q" + tg, name="q" + tg)
        nc.scalar.activation(q[:], s[:], AF.Identity,
                             bias=dsb[:, 0:1], scale=dsb[:, 1:2])
        rq = work.tile([128, M], F32, tag="rq" + tg, name="rq" + tg)
        nc.vector.tensor_mul(rq[:], r[:], q[:])
        nc.vector.scalar_tensor_tensor(out_sb[:, M:2 * M], ss[:], dsb[:, 2:3],
                                       rq[:], OP.mult, OP.add)

        # var = (1 - erho) * var0 + tau = -((erho - 1) * var0) + tau
        t1 = work.tile([128, M], F32, tag="t1" + tg, name="t1" + tg)
        nc.vector.scalar_tensor_tensor(t1[:], erho[:], 1.0, var0, OP.subtract,
                                       OP.mult)
        nc.vector.tensor_scalar(out_sb[:, 2 * M:3 * M], t1[:], -1.0, TAU_INV,
                                OP.mult, OP.add)

        # pi = epi / sum(epi)
        ssum = work.tile([128, 1], F32, tag="ssum" + tg, name="ssum" + tg)
        nc.vector.tensor_reduce(ssum[:], epi, mybir.AxisListType.X, OP.add)
        rsum = work.tile([128, 1], F32, tag="rsum" + tg, name="rsum" + tg)
        nc.vector.reciprocal(rsum[:], ssum[:])
        nc.vector.tensor_scalar_mul(out_sb[:, 0:M], epi, rsum[:])

        nc.sync.dma_start(out_dram[t * 128:(t + 1) * 128, :], out_sb[:])


def _declare_io(nc):
    # fwc: per contraction chunk, batch-tile-0 features fused with the weight
    # block (one DMA -> one matmul wait, and tile 0's inputs finish ~0.5MB of
    # DMA earlier than tile 1's, so its epilogue overlaps tile 1's loads).
    # ft1: tile-1 features, loaded last as two contiguous-run DMAs.
    fwc_dram = nc.dram_tensor("fwc", [KC, 128, 128 + NW], MM_DT,
                              kind="ExternalInput").ap()
    ft1_dram = nc.dram_tensor("ft1", [128, KC, 128], MM_DT,
                              kind="ExternalInput").ap()
    blk_dram = nc.dram_tensor("bias_blk", [1, 128 + NW], MM_DT,
                              kind="ExternalInput").ap()
    out_dram = nc.dram_tensor("out", [BC, 3 * M], F32, kind="ExternalOutput").ap()
    return fwc_dram, ft1_dram, blk_dram, out_dram


def _warmup_act(nc, consts):
    # Trigger the ACT exp/tanh table load immediately, overlapping the
    # input DMAs (it costs ~2.7us once per kernel).
    warm_in = consts.tile([128, 1], F32, tag="warm_in", name="warm_in")
    warm_out = consts.tile([128, 1], F32, tag="warm_out", name="warm_out")
    nc.vector.memset(warm_in[:], 0.0)
    nc.scalar.activation(warm_out[:], warm_in[:], AF.Exp)


def _warmup_pe(nc, consts, psum, n_fillers=9):
    # The PE HAM clock-gate only unthrottles (1.2 -> 2.4 GHz) after ~3.4us of
    # sustained activity. Feed it scratch matmuls while the input DMAs stream
    # so the real (dependency-gated) matmuls run at full clock.
    wsrc = consts.tile([1, 128], MM_DT, tag="pe_w", name="pe_w")
    nc.vector.memset(wsrc[:], 1.0)
    msrc = consts.tile([1, 512], MM_DT, tag="pe_m", name="pe_m")
    nc.vector.memset(msrc[:], 1.0)
    scratch = psum.tile([128, 512], F32, tag="pe_scratch", name="pe_scratch",
                        bufs=1)
    for i in range(n_fillers):
        nc.tensor.matmul(scratch[:], wsrc[:], msrc[:], start=True, stop=True)


def _build_nc():
    nc = bacc.Bacc("TRN2", target_bir_lowering=False, debug=False)
    fwc_dram, ft1_dram, blk_dram, out_dram = _declare_io(nc)
    with tile.TileContext(nc) as tc, ExitStack() as ctx:
        consts = ctx.enter_context(tc.tile_pool(name="consts", bufs=1))
        fwpool = ctx.enter_context(tc.tile_pool(name="fw", bufs=1))
        psum = ctx.enter_context(tc.tile_pool(name="psum", bufs=NT, space="PSUM"))
        work = ctx.enter_context(tc.tile_pool(name="work", bufs=NT))
        _warmup_act(nc, consts)
        _warmup_pe(nc, consts, psum)
        _emit_body(nc, tc, (consts, fwpool, psum, work),
                   fwc_dram, ft1_dram, blk_dram, out_dram)
    nc.compile()
    return nc


def build_loop_nc(reps):
    """Timing variant: run the body `reps` times inside one NEFF (used only
    by the local test harness; the default full-barrier back-edge keeps
    iterations serialized so per-iter span ~ single-shot kernel time)."""
    nc = bacc.Bacc("TRN2", target_bir_lowering=False, debug=False)
    fwc_dram, ft1_dram, blk_dram, out_dram = _declare_io(nc)
    with tile.TileContext(nc) as tc, ExitStack() as ctx:
        consts = ctx.enter_context(tc.tile_pool(name="consts", bufs=1))
        fwpool = ctx.enter_context(tc.tile_pool(name="fw", bufs=1))
        psum = ctx.enter_context(tc.tile_pool(name="psum", bufs=NT, space="PSUM"))
        work = ctx.enter_context(tc.tile_pool(name="work", bufs=NT))
        _warmup_act(nc, consts)
        with tc.For_i(0, reps, 1):
            _warmup_pe(nc, consts, psum)
            _emit_body(nc, tc, (consts, fwpool, psum, work),
                       fwc_dram, ft1_dram, blk_dram, out_dram)
    nc.compile()
    return nc


_CACHE = {}


def _get_nc():
    if "nc" not in _CACHE:
        _CACHE["nc"] = _build_nc()
    return _CACHE["nc"]


def _host_prep(inputs):
    f32 = np.float32
    feature = np.ascontiguousarray(inputs["feature"], dtype=f32)
    muW = np.asarray(inputs["muW"], dtype=f32)
    W = np.asarray(inputs["W"], dtype=f32)
    Z = np.asarray(inputs["Z"], dtype=f32)
    logvarW = np.asarray(inputs["logvarW"], dtype=f32)
    logvarZ = np.asarray(inputs["logvarZ"], dtype=f32)

    wstd = np.sqrt(np.exp(logvarW)).astype(f32)
    zstd = np.sqrt(np.exp(logvarZ)).astype(f32)
    a = ((zstd / wstd).astype(f32) * (W - muW)).astype(f32)
    v3 = np.stack([muW, a, Z, np.zeros_like(muW)], axis=1)  # [F, 4]

    wcat = np.concatenate(
        [-np.asarray(inputs["h2rho_w"], dtype=f32).T,
         np.asarray(inputs["h2pi_w"], dtype=f32).T,
         np.asarray(inputs["h2var_w"], dtype=f32).T,
         v3],
        axis=1,
    )  # [F, 387]
    wcat = wcat.reshape(KC, 128, NW)

    bias_blk = np.concatenate(
        [np.ones(128, dtype=f32),
         -np.asarray(inputs["h2rho_b"], dtype=f32),
         np.asarray(inputs["h2pi_b"], dtype=f32),
         np.asarray(inputs["h2var_b"], dtype=f32),
         np.zeros(4, dtype=f32)],
    ).reshape(1, 128 + NW)
    bias_blk = np.ascontiguousarray(bias_blk)

    in_maps = []
    for c in range(NCORES):
        shard = feature[c * BC:(c + 1) * BC]            # [BC, F]
        featT = shard.T.reshape(KC, 128, NT, 128)       # [c, p, half, j]
        fwc = np.ascontiguousarray(
            np.concatenate([featT[:, :, 0, :], wcat], axis=2),
            dtype=MM_NP)                                # [KC,128,128+NW]
        ft1 = np.ascontiguousarray(
            featT[:, :, 1, :].transpose(1, 0, 2), dtype=MM_NP)  # [128(p),KC,128]
        in_maps.append({"fwc": fwc, "ft1": ft1,
                        "bias_blk": bias_blk.astype(MM_NP)})
    return in_maps


def kernel(**inputs):
    nc = _get_nc()
    in_maps = _host_prep(inputs)
    res = run_bass_kernel_spmd(nc, in_maps, list(range(NCORES)))
    full = np.concatenate([res.results[c]["out"] for c in range(NCORES)], axis=0)
    pi = np.ascontiguousarray(full[:, 0:M])
    mu = np.ascontiguousarray(full[:, M:2 * M])
    var = np.ascontiguousarray(full[:, 2 * M:3 * M])
    return pi, mu, var



# revision 13
# speedup vs baseline: 1.7581x; 1.7581x over previous
"""Trainium2 Bass kernel for nn_MDCN (mixture-density head forward pass).

Reference computation (B=2048, F=1024, M=128):
    rho = tanh(feature @ h2rho_w.T + h2rho_b);  rho[:, 0] = 0.95
    pi  = softmax(feature @ h2pi_w.T + h2pi_b)
    var0 = exp(feature @ h2var_w.T + h2var_b)
    var = (1 - exp(rho)) * var0 + 1e-4
    W_ = r*muW + s*(r*(zstd/wstd)*(W-muW) + Z*s),  s = sqrt(1-r^2)
    mu = einsum('bmf,bf->bm', W_, feature)

Algebraic collapse: with a = (zstd/wstd)*(W-muW),
    mu[b,m] = r*d1[b] + r*s*d2[b] + (1-r^2)*d3[b]
            = r*(d1 + s*d2 - r*d3) + d3
  where d1 = feature@muW, d2 = feature@a, d3 = feature@Z.  The four columns
  [muW | a | -Z | Z] ride along in one fused matmul, so the whole epilogue is
  elementwise with per-row scalars and needs no extra matmuls.  s is computed
  as (1+r)*exp(-u) = sqrt(1-r^2), so one wide Exp + one Tanh + one Exp(r)
  cover all transcendentals (single ACT table set).

The clamped first mixture (rho[:,0] = 0.95) is baked into the matmul: the
rho-weight column 0 is zeroed and its bias set to atanh(0.95), so tanh
reproduces the constant and no memset/fixup instructions are needed.

Sharding: pure data-parallel over batch across 8 cores (256 rows/core =
2 partition tiles), weights replicated, no collectives.  Per-core DMA
sustains ~310 GB/s regardless of queue count (measured), so the input is a
single fused stream [f_tile0 | f_tile1 | weights] per 128-contraction chunk,
split into 4 DMAs so matmuls trail the stream.  Both PSUM tiles land in
adjacent banks and the epilogue processes them together with [128,2,*] APs
(halves instruction-count overhead); work tiles are fp16 for 2x DVE
throughput.  Outputs are written fp16 (host upcasts) in two pieces so the
second out-DMA's descriptor-gen overlaps the first's transfer.
"""

import os
from contextlib import ExitStack

import numpy as np

import concourse.bass as bass
import concourse.bacc as bacc
import concourse.mybir as mybir
import concourse.tile as tile
from concourse.bass_utils import run_bass_kernel_spmd

B, F, M = 2048, 1024, 128
NCORES = 8
BC = B // NCORES            # 256 batch rows per core
NT = BC // 128              # 2 partition tiles per core
KC = F // 128               # 8 contraction chunks
NW = 3 * M + 4              # 388 fused output cols: 384 logits + [d1 d2 -d3 d3]
CW = 2 * 128 + NW           # 644 fused stream cols per chunk: [f0 | f1 | w]
PW = 512                    # psum tile pitch (f32 words) = one full bank
NG = 4                      # input stream DMA groups (2 chunks each)
RHO_1 = 0.95
TAU_INV = 1.0e-4
ATANH_RHO1 = float(np.arctanh(np.float32(RHO_1)))

F32 = mybir.dt.float32
F16 = mybir.dt.float16
AF = mybir.ActivationFunctionType
OP = mybir.AluOpType
AX = mybir.AxisListType

MM_DT = F16                 # matmul operand dtype (DMA-bandwidth bound)
WK_DT = F16                 # epilogue work dtype (2x DVE throughput)
MM_NP = np.float16

# PE warm-up fillers: keep the PE busy while the input streams so the HAM
# clock-gate (1.2 -> 2.4 GHz after ~3us of activity) is unthrottled by the
# time the real matmuls run.  N_PRE run before the first chunk matmuls; N_GAP
# after each chunk pair covers any inter-DMA-group gaps.
N_PRE = int(os.environ.get("MDCN_N_PRE", "8"))
N_GAP = int(os.environ.get("MDCN_N_GAP", "0"))
FILL_W = 256
# Input stream DMA grouping (chunks per dma_start, must sum to KC).  Finer
# tail groups shrink the last consumer-semaphore wait; coarser head groups
# keep the HWDGE descriptor-gen (~625ns each) off the critical path.
GROUPS = [int(x) for x in os.environ.get("MDCN_GROUPS", "2,2,2,1,1").split(",")]


def _declare_io(nc):
    fwc_d = nc.dram_tensor("fwc", [128, KC, CW], MM_DT, kind="ExternalInput").ap()
    blk_d = nc.dram_tensor("bias_blk", [1, 128 + NW], MM_DT,
                           kind="ExternalInput").ap()
    opv_d = nc.dram_tensor("opv", [128, NT, 2 * M], F16,
                           kind="ExternalOutput").ap()
    omu_d = nc.dram_tensor("omu", [128, NT, M], F16, kind="ExternalOutput").ap()
    return fwc_d, blk_d, opv_d, omu_d


def _warmup_act(nc, consts):
    # Trigger the ACT table load (exp_and_others covers Exp+Tanh) right away,
    # overlapping the input DMAs (~1.3us once per kernel).
    wi = consts.tile([128, 1], F32, tag="warm_in", name="warm_in")
    wo = consts.tile([128, 1], F32, tag="warm_out", name="warm_out")
    nc.vector.memset(wi[:], 0.0)
    nc.scalar.activation(wo[:], wi[:], AF.Exp)
    nc.scalar.activation(wo[:], wi[:], AF.Tanh)


def _warmup_pe_srcs(nc, consts):
    wsrc = consts.tile([1, 128], MM_DT, tag="pe_w", name="pe_w")
    nc.vector.memset(wsrc[:], 1.0)
    msrc = consts.tile([1, FILL_W], MM_DT, tag="pe_m", name="pe_m")
    nc.vector.memset(msrc[:], 1.0)
    return wsrc, msrc


def _emit_body(nc, tc, pools, io):
    consts, stream, psum, work = pools
    fwc_d, blk_d, opv_d, omu_d = io

    # Tiny bias block on the SWDGE (gpsimd) queue so it doesn't delay the
    # head of the SP queue that streams the big fused input.
    blk = consts.tile([1, 128 + NW], MM_DT, tag="bias_blk", name="bias_blk")
    nc.gpsimd.dma_start(blk[:], blk_d)

    # Fused input stream in contraction order on one queue (queue-splitting
    # does not increase per-core DMA bandwidth on this part).
    assert sum(GROUPS) == KC
    chunk_tile = {}
    off = 0
    for i, gsz in enumerate(GROUPS):
        g = stream.tile([128, gsz, CW], MM_DT, tag=f"g{i}", name=f"g{i}")
        nc.sync.dma_start(g[:], fwc_d[:, off:off + gsz, :])
        for j in range(gsz):
            chunk_tile[off + j] = (g, j)
        off += gsz

    # Both batch tiles' accumulators side by side in adjacent PSUM banks.
    P = psum.tile([128, NT, PW], F32, tag="P", name="P")

    wsrc, msrc = _warmup_pe_srcs(nc, consts)
    scratch = psum.tile([128, FILL_W], F32, tag="pe_scratch", name="pe_scratch",
                        bufs=1)

    def fill(n):
        for _ in range(n):
            nc.tensor.matmul(scratch[:], wsrc[:], msrc[:], start=True, stop=True)

    fill(N_PRE)
    # c0 takes start=True (zeroing); the bias matmuls slide in mid-stream
    # (after chunk 3) where the PE has slack, so the bias DMA never stalls
    # the pipeline start and adds nothing to the tail.
    for c in range(KC):
        gt, s = chunk_tile[c]
        for t in range(NT):
            nc.tensor.matmul(P[:, t, 0:NW], gt[:, s, 128 * t:128 * (t + 1)],
                             gt[:, s, 256:256 + NW],
                             start=(c == 0), stop=(c == KC - 1))
        if c == 3:
            for t in range(NT):
                nc.tensor.matmul(P[:, t, 0:NW], blk[:, 0:128],
                                 blk[:, 128:128 + NW], start=False, stop=False)
        if N_GAP and c % 2 == 1 and c < KC - 1:
            fill(N_GAP)

    # ---- epilogue, both tiles at once via [128, NT, *] APs ----
    # psum cols: [0:M] = -u (rho), [M:2M] = pi logits, [2M:3M] = var logits,
    #            [3M:3M+4] = [d1, d2, -d3, d3]
    # ACT order: [e^-u | e^pi] first (unblocks the mu chain AND the pi sum
    # early), tanh, e^var, e^r last (var chain is shorter than the mu chain).
    E01 = work.tile([128, NT, 2 * M], WK_DT, tag="E01", name="E01")
    nc.scalar.activation(E01[:], P[:, :, 0:2 * M], AF.Exp)  # e^-u | e^pi
    r = work.tile([128, NT, M], WK_DT, tag="r", name="r")
    nc.scalar.activation(r[:], P[:, :, 0:M], AF.Tanh, scale=-1.0)
    ev = work.tile([128, NT, M], WK_DT, tag="ev", name="ev")
    nc.scalar.activation(ev[:], P[:, :, 2 * M:3 * M], AF.Exp)  # var0
    erho = work.tile([128, NT, M], WK_DT, tag="erho", name="erho")
    nc.scalar.activation(erho[:], r[:], AF.Exp)

    en, epi = E01[:, :, 0:M], E01[:, :, M:2 * M]

    # per-row scalars staged into SBUF (f32): HW scalar-ptr operands must not
    # read PSUM (device faults), and the mult rule requires f32 scalars.
    dsb = work.tile([128, NT, 4], F32, tag="dsb", name="dsb")
    nc.vector.tensor_copy(dsb[:], P[:, :, 3 * M:3 * M + 4])

    def dcol(t, j):
        return dsb[:, t, j:j + 1]

    # pi: sums in f32 (internal accumulation), scale epi by 1/sum per tile
    sums = work.tile([128, NT], F32, tag="sums", name="sums")
    nc.vector.tensor_reduce(sums[:], epi, AX.X, OP.add)
    rs = work.tile([128, NT], F32, tag="rs", name="rs")
    nc.vector.reciprocal(rs[:], sums[:])

    opv = work.tile([128, NT, 2 * M], F16, tag="opv", name="opv")
    omu = work.tile([128, NT, M], F16, tag="omu", name="omu")

    for t in range(NT):
        nc.vector.tensor_scalar_mul(opv[:, t, 0:M], E01[:, t, M:2 * M],
                                    rs[:, t:t + 1])

    # s = (1 + r) * e^-u = sqrt(1 - r^2)
    s_t = work.tile([128, NT, M], WK_DT, tag="s", name="s")
    nc.vector.scalar_tensor_tensor(s_t[:], r[:], 1.0, en, OP.add, OP.mult)
    # mu = r*(d1 + s*d2 - r*d3) + d3
    w1 = work.tile([128, NT, M], WK_DT, tag="w1", name="w1")
    w2 = work.tile([128, NT, M], WK_DT, tag="w2", name="w2")
    for t in range(NT):
        nc.vector.tensor_scalar(w1[:, t, :], s_t[:, t, :], dcol(t, 1),
                                dcol(t, 0), OP.mult, OP.add)
    for t in range(NT):
        nc.vector.scalar_tensor_tensor(w2[:, t, :], r[:, t, :], dcol(t, 2),
                                       w1[:, t, :], OP.mult, OP.add)
    w3 = work.tile([128, NT, M], WK_DT, tag="w3", name="w3")
    nc.vector.tensor_mul(w3[:], r[:], w2[:])
    for t in range(NT):
        nc.vector.tensor_scalar_add(omu[:, t, :], w3[:, t, :], dcol(t, 3))
    nc.sync.dma_start(omu_d, omu[:])

    # var = (1 - e^r) * var0 + tau = -((e^r - 1) * var0) + tau
    t1 = work.tile([128, NT, M], WK_DT, tag="t1", name="t1")
    nc.vector.scalar_tensor_tensor(t1[:], erho[:], 1.0, ev, OP.subtract, OP.mult)
    nc.vector.tensor_scalar(opv[:, :, M:2 * M], t1[:], -1.0, TAU_INV,
                            OP.mult, OP.add)
    nc.sync.dma_start(opv_d, opv[:])


def _pools(tc, ctx):
    consts = ctx.enter_context(tc.tile_pool(name="consts", bufs=1))
    stream = ctx.enter_context(tc.tile_pool(name="stream", bufs=1))
    psum = ctx.enter_context(tc.tile_pool(name="psum", bufs=1, space="PSUM"))
    work = ctx.enter_context(tc.tile_pool(name="work", bufs=1))
    return consts, stream, psum, work


def _build_nc():
    nc = bacc.Bacc("TRN2", target_bir_lowering=False, debug=False)
    io = _declare_io(nc)
    with tile.TileContext(nc) as tc, ExitStack() as ctx:
        pools = _pools(tc, ctx)
        _warmup_act(nc, pools[0])
        _emit_body(nc, tc, pools, io)
    nc.compile()
    return nc


def build_loop_nc(reps):
    """Timing variant: run the body `reps` times inside one NEFF (full-barrier
    back-edge serializes iterations => per-iter span ~ single-shot time)."""
    nc = bacc.Bacc("TRN2", target_bir_lowering=False, debug=False)
    io = _declare_io(nc)
    with tile.TileContext(nc) as tc, ExitStack() as ctx:
        pools = _pools(tc, ctx)
        _warmup_act(nc, pools[0])
        with tc.For_i(0, reps, 1):
            _emit_body(nc, tc, pools, io)
    nc.compile()
    return nc


_CACHE = {}


def _get_nc():
    if "nc" not in _CACHE:
        _CACHE["nc"] = _build_nc()
    return _CACHE["nc"]


def _host_prep(inputs):
    f32 = np.float32
    feature = np.ascontiguousarray(inputs["feature"], dtype=f32)
    muW = np.asarray(inputs["muW"], dtype=f32)
    W = np.asarray(inputs["W"], dtype=f32)
    Z = np.asarray(inputs["Z"], dtype=f32)
    logvarW = np.asarray(inputs["logvarW"], dtype=f32)
    logvarZ = np.asarray(inputs["logvarZ"], dtype=f32)

    wstd = np.sqrt(np.exp(logvarW)).astype(f32)
    zstd = np.sqrt(np.exp(logvarZ)).astype(f32)
    a = ((zstd / wstd).astype(f32) * (W - muW)).astype(f32)
    v4 = np.stack([muW, a, -Z, Z], axis=1)                # [F, 4]

    rho_w = -np.asarray(inputs["h2rho_w"], dtype=f32).T   # [F, M], negated
    rho_w[:, 0] = 0.0                                     # clamped mixture
    wcat = np.concatenate(
        [rho_w,
         np.asarray(inputs["h2pi_w"], dtype=f32).T,
         np.asarray(inputs["h2var_w"], dtype=f32).T,
         v4],
        axis=1,
    )                                                     # [F, 388]

    brho = -np.asarray(inputs["h2rho_b"], dtype=f32)
    brho[0] = -ATANH_RHO1                                 # tanh(-(-atanh)) = .95
    bias_blk = np.concatenate(
        [np.ones(128, dtype=f32), brho,
         np.asarray(inputs["h2pi_b"], dtype=f32),
         np.asarray(inputs["h2var_b"], dtype=f32),
         np.zeros(4, dtype=f32)],
    ).reshape(1, 128 + NW).astype(MM_NP)

    wc = wcat.reshape(KC, 128, NW)                        # [c, k, j]
    in_maps = []
    for c in range(NCORES):
        shard = feature[c * BC:(c + 1) * BC]              # [BC, F]
        featT = shard.T.reshape(KC, 128, NT, 128)         # [c, k, t, b]
        fused = np.concatenate([featT[:, :, 0, :], featT[:, :, 1, :], wc],
                               axis=2)                    # [c, k, 644]
        fwc = np.ascontiguousarray(fused.transpose(1, 0, 2), dtype=MM_NP)
        in_maps.append({"fwc": fwc, "bias_blk": bias_blk})
    return in_maps


def _unpack(res, cores):
    pi = np.empty((B, M), dtype=np.float32)
    mu = np.empty((B, M), dtype=np.float32)
    var = np.empty((B, M), dtype=np.float32)
    for c in cores:
        pv = np.asarray(res[c]["opv"]).transpose(1, 0, 2).reshape(BC, 2 * M)
        m_ = np.asarray(res[c]["omu"]).transpose(1, 0, 2).reshape(BC, M)
        sl = slice(c * BC, (c + 1) * BC)
        pi[sl] = pv[:, 0:M]
        var[sl] = pv[:, M:2 * M]
        mu[sl] = m_
    return pi, mu, var


def kernel(**inputs):
    nc = _get_nc()
    in_maps = _host_prep(inputs)
    res = run_bass_kernel_spmd(nc, in_maps, list(range(NCORES)))
    return _unpack(res.results, list(range(NCORES)))
